# revision 3
# baseline (speedup 1.0000x reference)
"""Distributed exact inner-product top-k (brute-force kNN) on 8 TRN2 NeuronCores.

Layout per core i (vocab shard of 25000 rows of W):
  - wt  [128, 25000] f32 : W[25000*i : 25000*(i+1)].T  (host pre-transposed)
  - xt  [128, 1024]  f32 : x.T (replicated)
Device kernel (SPMD, identical graph, no collectives):
  - scores tile [128 rows, 512 vocab] = matmul(xT_tile, wt_chunk) in PSUM (f32r
    path: exact fp32 multiply-accumulate at 1 cycle/row)
  - per-chunk top-8 values + indices via DVE max / max_index
  - DMA per-chunk candidates to DRAM
Host merge:
  - global top-128 per row from the 8*49*8 = 3136 candidates, with the jax
    tie order (descending value, ascending index)
  - exactness guard: a 512-wide chunk can hide a top-128 element only if its
    8th-best candidate still clears the row's final 128th value; those rows
    (probability ~1e-9 per run for N(0,1) data) are recomputed exactly on host.
"""

import os

import numpy as np

import concourse.bass as bass  # noqa: F401  (bass must be imported before tile)
import concourse.tile as tile
from concourse import bacc, mybir
from concourse.bass_utils import run_bass_kernel_spmd

B = 1024
D = 128
VOCAB = 200000
NCORES = 8
VSHARD = VOCAB // NCORES  # 25000
CHUNK = 512
NCHUNK = (VSHARD + CHUNK - 1) // CHUNK  # 49 (last chunk 424)
NCAND = NCHUNK * 8  # 392
TOPK = 128

F32 = mybir.dt.float32
F32R = mybir.dt.float32r
U32 = mybir.dt.uint32

LAST_RESULTS = None  # BassKernelResults of the most recent run (for profiling)
_CACHED_NC = None


def build_kernel():
    nc = bacc.Bacc("TRN2", target_bir_lowering=False, debug=False)
    wt_d = nc.dram_tensor("wt", [D, VSHARD], F32R, kind="ExternalInput")
    xt_d = nc.dram_tensor("xt", [D, B], F32R, kind="ExternalInput")
    vals_d = nc.dram_tensor("out_vals", [B, NCAND], F32, kind="ExternalOutput")
    idx_d = nc.dram_tensor("out_idx", [B, NCAND], U32, kind="ExternalOutput")

    with tile.TileContext(nc) as tc:
        with (
            tc.tile_pool(name="wt", bufs=1) as wt_pool,
            tc.tile_pool(name="xt", bufs=1) as xt_pool,
            tc.tile_pool(name="psum", bufs=8, space="PSUM") as psum_pool,
            tc.tile_pool(name="cand", bufs=2) as cand_pool,
        ):
            wt_sb = wt_pool.tile([D, VSHARD], F32R)
            xt_sb = xt_pool.tile([D, B], F32R)
            # Split the big W DMA so several queues run in parallel.
            nsplit = 8
            step = VSHARD // nsplit
            for s in range(nsplit):
                hi = VSHARD if s == nsplit - 1 else (s + 1) * step
                nc.sync.dma_start(wt_sb[:, s * step:hi], wt_d[:, s * step:hi])
            nc.sync.dma_start(xt_sb[:], xt_d[:])

            for g in range(B // 128):
                vals_sb = cand_pool.tile([128, NCAND], F32, tag="vals")
                idx_sb = cand_pool.tile([128, NCAND], U32, tag="idx")
                for c in range(NCHUNK):
                    w = min(CHUNK, VSHARD - c * CHUNK)
                    ps = psum_pool.tile([128, CHUNK], F32)
                    nc.tensor.matmul(
                        ps[:, :w],
                        xt_sb[:, g * 128:(g + 1) * 128],
                        wt_sb[:, c * CHUNK:c * CHUNK + w],
                        start=True, stop=True,
                    )
                    nc.vector.max(vals_sb[:, 8 * c:8 * c + 8], ps[:, :w])
                    nc.vector.max_index(
                        idx_sb[:, 8 * c:8 * c + 8],
                        vals_sb[:, 8 * c:8 * c + 8],
                        ps[:, :w],
                    )
                nc.sync.dma_start(vals_d[g * 128:(g + 1) * 128, :], vals_sb[:])
                nc.sync.dma_start(idx_d[g * 128:(g + 1) * 128, :], idx_sb[:])
    nc.compile()
    return nc


def _topk_rows(vals, gidx, k):
    """Per-row top-k of (vals desc, gidx asc) -> index array [rows, k]."""
    order = np.lexsort(
        (gidx, -vals.astype(np.float64)), axis=-1
    )[:, :k]
    return np.take_along_axis(gidx, order, axis=1), np.take_along_axis(
        vals, order, axis=1
    )


# The PE's fast-fp32 path (float32r) carries a reduced-precision multiply:
# measured |err| <= ~0.01 on scores of magnitude ~72.  Selection margin used
# by the exactness guard below (5x the measured worst case).
F32R_NOISE_BOUND = 0.05


def kernel(x: np.ndarray, W: np.ndarray, topk) -> np.ndarray:
    global LAST_RESULTS, _CACHED_NC
    assert x.shape == (B, D) and W.shape == (VOCAB, D)
    assert int(topk) == TOPK
    x = np.ascontiguousarray(np.asarray(x, dtype=np.float32))
    W = np.ascontiguousarray(np.asarray(W, dtype=np.float32))

    if _CACHED_NC is None:
        _CACHED_NC = build_kernel()
    nc = _CACHED_NC

    xt = np.ascontiguousarray(x.T)
    in_maps = []
    for i in range(NCORES):
        wt_i = np.ascontiguousarray(W[i * VSHARD:(i + 1) * VSHARD].T)
        in_maps.append({"wt": wt_i, "xt": xt})

    LAST_RESULTS = run_bass_kernel_spmd(
        nc,
        in_maps,
        core_ids=list(range(NCORES)),
        trace=bool(int(os.environ.get("KERNEL_TRACE", "0"))),
    )
    results = LAST_RESULTS.results

    vals_all = np.concatenate(
        [results[i]["out_vals"] for i in range(NCORES)], axis=1
    )  # [B, 8*392]
    idx_local = np.concatenate(
        [results[i]["out_idx"].astype(np.int64) for i in range(NCORES)], axis=1
    )
    # local chunk idx -> global vocab idx
    chunk_base = np.concatenate(
        [
            i * VSHARD + CHUNK * (np.arange(NCAND) // 8)
            for i in range(NCORES)
        ]
    ).astype(np.int64)  # [8*392]
    gidx_all = idx_local + chunk_base[None, :]

    # Exact re-rank of the device-selected candidates: f64 inner products of
    # the ~3136 candidates per row (0.8 GFLOP on host vs 52 GFLOP on device).
    x64 = x.astype(np.float64)
    W64 = W.astype(np.float64)
    exact = np.empty_like(vals_all, dtype=np.float64)
    BATCH = 128
    for r0 in range(0, B, BATCH):
        r1 = r0 + BATCH
        gW = W64[gidx_all[r0:r1]]  # [BATCH, ncand, D]
        exact[r0:r1] = np.einsum("bjd,bd->bj", gW, x64[r0:r1])

    gidx_top, vals_top = _topk_rows(exact, gidx_all, TOPK)

    # Exactness guard: a chunk can hide a true top-128 element only if its
    # 8th-best (by noisy device score) is still within the noise margin of
    # the row's exact 128th value.  Duplicate winners (max_index tie
    # pathology) also invalidate a row.
    t_row = vals_top[:, -1]  # [B]
    chunk_min = vals_all.reshape(B, -1, 8)[:, :, 7].astype(np.float64)
    risky = (chunk_min >= (t_row[:, None] - F32R_NOISE_BOUND)).any(axis=1)
    idx_chunks = np.sort(gidx_all.reshape(B, -1, 8), axis=2)
    dup = (np.diff(idx_chunks, axis=2) == 0).any(axis=(1, 2))
    bad = np.flatnonzero(risky | dup)
    for r in bad:
        s = x64[r] @ W64.T
        order = np.lexsort((np.arange(VOCAB), -s))[:TOPK]
        gidx_top[r] = order

    return gidx_top.astype(np.int32)


# revision 4
# speedup vs baseline: 1.0323x; 1.0323x over previous
"""Distributed exact inner-product top-k (brute-force kNN) on 8 TRN2 NeuronCores.

Sharding: codebook W is split row-wise into 8 shards of 25000 (one per core);
x is replicated.  Host pre-transposes both so the contraction dim (128) lands
on SBUF partitions.

Device kernel (SPMD, identical graph per core, no collectives needed):
  - per 512-wide vocab chunk: scores tile [128 rows, 512] = bf16 matmul into
    PSUM (f32 accumulation)
  - DVE max / max_index extract the chunk's top-8 values + positions
  - candidates (49 chunks x 8 = 392 per row per core) DMA'd out

Host merge (the all-gather + final top-k of the distributed ANN pattern):
  - exact f64 re-rank of the 8*392 = 3136 device-selected candidates per row
    (0.8 GFLOP on host vs 52 GFLOP of scoring on device) removes the bf16/
    fast-matmul selection noise entirely
  - final top-128 ordered like jax.lax.top_k (value desc, index asc)
  - exactness guard: a 512-chunk can hide a true top-128 element only if its
    8th-best device score clears the row's exact 128th value minus the score
    noise bound; such rows (expected ~0 per run for this data distribution)
    are recomputed exactly on host, as are rows with duplicated winners.
"""

import numpy as np

B = 1024
D = 128
VOCAB = 200000
NCORES = 8
VSHARD = VOCAB // NCORES  # 25000
CHUNK = 512
NCHUNK = (VSHARD + CHUNK - 1) // CHUNK  # 49 (last chunk is 424 wide)
NCAND = NCHUNK * 8  # 392
TOPK = 128

# Device scores use bf16 inputs (f32 accumulation): |device - exact| on scores
# of scale ~72 measured < 0.2; guard margin is ~2.5x that worst case.
SCORE_NOISE_BOUND = 0.5

LAST_RESULTS = None  # BassKernelResults of the most recent run (for profiling)
_CACHED_NC = None


def build_kernel():
    import concourse.bass as bass  # noqa: F401
    import concourse.tile as tile
    from concourse import bacc, mybir

    F32 = mybir.dt.float32
    BF16 = mybir.dt.bfloat16
    U32 = mybir.dt.uint32

    nc = bacc.Bacc("TRN2", target_bir_lowering=False, debug=False)
    wt_d = nc.dram_tensor("wt", [D, VSHARD], BF16, kind="ExternalInput")
    xt_d = nc.dram_tensor("xt", [D, B], BF16, kind="ExternalInput")
    vals_d = nc.dram_tensor("out_vals", [B, NCAND], F32, kind="ExternalOutput")
    idx_d = nc.dram_tensor("out_idx", [B, NCAND], U32, kind="ExternalOutput")

    with tile.TileContext(nc) as tc:
        with (
            tc.tile_pool(name="wt", bufs=1) as wt_pool,
            tc.tile_pool(name="xt", bufs=1) as xt_pool,
            tc.tile_pool(name="psum", bufs=8, space="PSUM") as psum_pool,
            tc.tile_pool(name="cand", bufs=2) as cand_pool,
        ):
            wt_sb = wt_pool.tile([D, VSHARD], BF16)
            xt_sb = xt_pool.tile([D, B], BF16)
            nsplit = 8
            step = VSHARD // nsplit
            for s in range(nsplit):
                hi = VSHARD if s == nsplit - 1 else (s + 1) * step
                nc.sync.dma_start(wt_sb[:, s * step:hi], wt_d[:, s * step:hi])
            nc.sync.dma_start(xt_sb[:], xt_d[:])

            for g in range(B // 128):
                vals_sb = cand_pool.tile([128, NCAND], F32, tag="vals")
                idx_sb = cand_pool.tile([128, NCAND], U32, tag="idx")
                for c in range(NCHUNK):
                    w = min(CHUNK, VSHARD - c * CHUNK)
                    ps = psum_pool.tile([128, CHUNK], F32)
                    nc.tensor.matmul(
                        ps[:, :w],
                        xt_sb[:, g * 128:(g + 1) * 128],
                        wt_sb[:, c * CHUNK:c * CHUNK + w],
                        start=True, stop=True,
                    )
                    nc.vector.max(vals_sb[:, 8 * c:8 * c + 8], ps[:, :w])
                    nc.vector.max_index(
                        idx_sb[:, 8 * c:8 * c + 8],
                        vals_sb[:, 8 * c:8 * c + 8],
                        ps[:, :w],
                    )
                nc.sync.dma_start(vals_d[g * 128:(g + 1) * 128, :], vals_sb[:])
                nc.sync.dma_start(idx_d[g * 128:(g + 1) * 128, :], idx_sb[:])
    nc.compile()
    return nc


def _topk_rows(vals, gidx, k):
    """Per-row top-k ordered like jax.lax.top_k: value desc, index asc."""
    order = np.lexsort((gidx, -vals), axis=-1)[:, :k]
    return (
        np.take_along_axis(gidx, order, axis=1),
        np.take_along_axis(vals, order, axis=1),
    )


def kernel(x: np.ndarray, W: np.ndarray, topk) -> np.ndarray:
    global LAST_RESULTS, _CACHED_NC
    import os

    import ml_dtypes

    from concourse.bass_utils import run_bass_kernel_spmd

    assert x.shape == (B, D) and W.shape == (VOCAB, D)
    assert int(topk) == TOPK
    x = np.ascontiguousarray(np.asarray(x, dtype=np.float32))
    W = np.ascontiguousarray(np.asarray(W, dtype=np.float32))

    if _CACHED_NC is None:
        _CACHED_NC = build_kernel()
    nc = _CACHED_NC

    xt = np.ascontiguousarray(x.T).astype(ml_dtypes.bfloat16)
    in_maps = []
    for i in range(NCORES):
        wt_i = np.ascontiguousarray(
            W[i * VSHARD:(i + 1) * VSHARD].T
        ).astype(ml_dtypes.bfloat16)
        in_maps.append({"wt": wt_i, "xt": xt})

    LAST_RESULTS = run_bass_kernel_spmd(
        nc,
        in_maps,
        core_ids=list(range(NCORES)),
        trace=bool(int(os.environ.get("KERNEL_TRACE", "0"))),
    )
    results = LAST_RESULTS.results

    vals_all = np.concatenate(
        [results[i]["out_vals"] for i in range(NCORES)], axis=1
    ).astype(np.float64)  # [B, 8*392]
    idx_local = np.concatenate(
        [results[i]["out_idx"].astype(np.int64) for i in range(NCORES)], axis=1
    )
    # per-chunk local index -> global vocab index
    chunk_base = np.concatenate(
        [i * VSHARD + CHUNK * (np.arange(NCAND) // 8) for i in range(NCORES)]
    ).astype(np.int64)  # [8*392]
    gidx_all = np.clip(idx_local, 0, CHUNK - 1) + chunk_base[None, :]
    bad_idx_rows = (idx_local >= CHUNK).any(axis=1)

    # Exact re-rank of device-selected candidates: f64 inner products.
    x64 = x.astype(np.float64)
    W64 = W.astype(np.float64)
    exact = np.empty_like(vals_all)
    STEP = 128
    for r0 in range(0, B, STEP):
        r1 = r0 + STEP
        gW = W64[gidx_all[r0:r1]]  # [STEP, ncand, D]
        exact[r0:r1] = np.einsum("bjd,bd->bj", gW, x64[r0:r1])

    gidx_top, vals_top = _topk_rows(exact, gidx_all, TOPK)

    # Exactness guard + fallback.
    t_row = vals_top[:, -1]  # [B] exact 128th value
    chunk_min = vals_all.reshape(B, -1, 8)[:, :, 7]
    risky = (chunk_min >= (t_row[:, None] - SCORE_NOISE_BOUND)).any(axis=1)
    idx_chunks = np.sort(gidx_all.reshape(B, -1, 8), axis=2)
    dup = (np.diff(idx_chunks, axis=2) == 0).any(axis=(1, 2))
    for r in np.flatnonzero(risky | dup | bad_idx_rows):
        s = x64[r] @ W64.T
        gidx_top[r] = np.lexsort((np.arange(VOCAB), -s))[:TOPK]

    return gidx_top.astype(np.int32)


# revision 5
# speedup vs baseline: 1.0614x; 1.0282x over previous
"""Distributed exact inner-product top-k (brute-force kNN) on 8 TRN2 NeuronCores.

Sharding: codebook W is split row-wise into 8 shards of 25000 (one per core);
x is replicated.  Host pre-transposes both so the contraction dim (128) lands
on SBUF partitions.

Device kernel (SPMD, identical graph per core, no collectives needed):
  - per 512-wide vocab chunk: scores tile [128 rows, 512] = bf16 matmul into
    PSUM (f32 accumulation)
  - DVE max / max_index extract the chunk's top-8 values + positions
  - candidates (49 chunks x 8 = 392 per row per core) DMA'd out

Host merge (the all-gather + final top-k of the distributed ANN pattern):
  - exact f64 re-rank of the 8*392 = 3136 device-selected candidates per row
    (0.8 GFLOP on host vs 52 GFLOP of scoring on device) removes the bf16/
    fast-matmul selection noise entirely
  - final top-128 ordered like jax.lax.top_k (value desc, index asc)
  - exactness guard: a 512-chunk can hide a true top-128 element only if its
    8th-best device score clears the row's exact 128th value minus the score
    noise bound; such rows (expected ~0 per run for this data distribution)
    are recomputed exactly on host, as are rows with duplicated winners.
"""

import numpy as np

B = 1024
D = 128
VOCAB = 200000
NCORES = 8
VSHARD = VOCAB // NCORES  # 25000
CHUNK = 512
NCHUNK = (VSHARD + CHUNK - 1) // CHUNK  # 49 (last chunk is 424 wide)
NCAND = NCHUNK * 8  # 392
TOPK = 128

# Device scores use bf16 inputs (f32 accumulation): |device - exact| on scores
# of scale ~72 measured < 0.2; guard margin is ~2.5x that worst case.
SCORE_NOISE_BOUND = 0.5

LAST_RESULTS = None  # BassKernelResults of the most recent run (for profiling)
_CACHED_NC = None


def build_kernel():
    import concourse.bass as bass  # noqa: F401
    import concourse.tile as tile
    from concourse import bacc, mybir

    F32 = mybir.dt.float32
    BF16 = mybir.dt.bfloat16
    U32 = mybir.dt.uint32

    nc = bacc.Bacc("TRN2", target_bir_lowering=False, debug=False)
    wt_d = nc.dram_tensor("wt", [D, VSHARD], BF16, kind="ExternalInput")
    xt_d = nc.dram_tensor("xt", [D, B], BF16, kind="ExternalInput")
    vals_d = nc.dram_tensor("out_vals", [B, NCAND], F32, kind="ExternalOutput")
    idx_d = nc.dram_tensor("out_idx", [B, NCAND], U32, kind="ExternalOutput")

    with tile.TileContext(nc) as tc:
        with (
            tc.tile_pool(name="wt", bufs=1) as wt_pool,
            tc.tile_pool(name="xt", bufs=1) as xt_pool,
            tc.tile_pool(name="psum", bufs=8, space="PSUM") as psum_pool,
            tc.tile_pool(name="cand", bufs=2) as cand_pool,
        ):
            wt_sb = wt_pool.tile([D, VSHARD], BF16)
            xt_sb = xt_pool.tile([D, B], BF16)
            # xt first: the first matmul's stationary operand should not wait
            # behind the whole 6.4MB W load; 16 splits spread W across queues.
            nc.sync.dma_start(xt_sb[:], xt_d[:])
            nsplit = 16
            step = VSHARD // nsplit
            for s in range(nsplit):
                hi = VSHARD if s == nsplit - 1 else (s + 1) * step
                nc.sync.dma_start(wt_sb[:, s * step:hi], wt_d[:, s * step:hi])

            for g in range(B // 128):
                vals_sb = cand_pool.tile([128, NCAND], F32, tag="vals")
                idx_sb = cand_pool.tile([128, NCAND], U32, tag="idx")
                for c in range(NCHUNK):
                    w = min(CHUNK, VSHARD - c * CHUNK)
                    ps = psum_pool.tile([128, CHUNK], F32)
                    nc.tensor.matmul(
                        ps[:, :w],
                        xt_sb[:, g * 128:(g + 1) * 128],
                        wt_sb[:, c * CHUNK:c * CHUNK + w],
                        start=True, stop=True,
                    )
                    nc.vector.max(vals_sb[:, 8 * c:8 * c + 8], ps[:, :w])
                    nc.vector.max_index(
                        idx_sb[:, 8 * c:8 * c + 8],
                        vals_sb[:, 8 * c:8 * c + 8],
                        ps[:, :w],
                    )
                nc.sync.dma_start(vals_d[g * 128:(g + 1) * 128, :], vals_sb[:])
                nc.sync.dma_start(idx_d[g * 128:(g + 1) * 128, :], idx_sb[:])
    nc.compile()
    return nc


def _topk_rows(vals, gidx, k):
    """Per-row top-k ordered like jax.lax.top_k: value desc, index asc."""
    order = np.lexsort((gidx, -vals), axis=-1)[:, :k]
    return (
        np.take_along_axis(gidx, order, axis=1),
        np.take_along_axis(vals, order, axis=1),
    )


def kernel(x: np.ndarray, W: np.ndarray, topk) -> np.ndarray:
    global LAST_RESULTS, _CACHED_NC
    import os

    import ml_dtypes

    from concourse.bass_utils import run_bass_kernel_spmd

    assert x.shape == (B, D) and W.shape == (VOCAB, D)
    assert int(topk) == TOPK
    x = np.ascontiguousarray(np.asarray(x, dtype=np.float32))
    W = np.ascontiguousarray(np.asarray(W, dtype=np.float32))

    if _CACHED_NC is None:
        _CACHED_NC = build_kernel()
    nc = _CACHED_NC

    xt = np.ascontiguousarray(x.T).astype(ml_dtypes.bfloat16)
    in_maps = []
    for i in range(NCORES):
        wt_i = np.ascontiguousarray(
            W[i * VSHARD:(i + 1) * VSHARD].T
        ).astype(ml_dtypes.bfloat16)
        in_maps.append({"wt": wt_i, "xt": xt})

    LAST_RESULTS = run_bass_kernel_spmd(
        nc,
        in_maps,
        core_ids=list(range(NCORES)),
        trace=bool(int(os.environ.get("KERNEL_TRACE", "0"))),
    )
    results = LAST_RESULTS.results

    vals_all = np.concatenate(
        [results[i]["out_vals"] for i in range(NCORES)], axis=1
    ).astype(np.float64)  # [B, 8*392]
    idx_local = np.concatenate(
        [results[i]["out_idx"].astype(np.int64) for i in range(NCORES)], axis=1
    )
    # per-chunk local index -> global vocab index
    chunk_base = np.concatenate(
        [i * VSHARD + CHUNK * (np.arange(NCAND) // 8) for i in range(NCORES)]
    ).astype(np.int64)  # [8*392]
    gidx_all = np.clip(idx_local, 0, CHUNK - 1) + chunk_base[None, :]
    bad_idx_rows = (idx_local >= CHUNK).any(axis=1)

    # Exact re-rank of device-selected candidates: f64 inner products.
    x64 = x.astype(np.float64)
    W64 = W.astype(np.float64)
    exact = np.empty_like(vals_all)
    STEP = 128
    for r0 in range(0, B, STEP):
        r1 = r0 + STEP
        gW = W64[gidx_all[r0:r1]]  # [STEP, ncand, D]
        exact[r0:r1] = np.einsum("bjd,bd->bj", gW, x64[r0:r1])

    gidx_top, vals_top = _topk_rows(exact, gidx_all, TOPK)

    # Exactness guard + fallback.
    t_row = vals_top[:, -1]  # [B] exact 128th value
    chunk_min = vals_all.reshape(B, -1, 8)[:, :, 7]
    risky = (chunk_min >= (t_row[:, None] - SCORE_NOISE_BOUND)).any(axis=1)
    idx_chunks = np.sort(gidx_all.reshape(B, -1, 8), axis=2)
    dup = (np.diff(idx_chunks, axis=2) == 0).any(axis=(1, 2))
    for r in np.flatnonzero(risky | dup | bad_idx_rows):
        s = x64[r] @ W64.T
        gidx_top[r] = np.lexsort((np.arange(VOCAB), -s))[:TOPK]

    return gidx_top.astype(np.int32)
